# revision 1
# baseline (speedup 1.0000x reference)
"""Binary depthwise 3x3 conv (SAME padding) on 8 Trainium2 NeuronCores.

Problem: x (16,112,112,384) f32, w (3,3,384,1) f32.
out[n,h,w,c] = sum_{dy,dx} sign(clip(w))[dy,dx,c] * x[n,h+dy-1,w+dx-1,c]

Strategy (data-parallel, 2 images per core):
  - DMA x in natural NHWC layout (contiguous per partition).
  - PE transpose-mode flips [spatial, c] -> [c, spatial] into a zero-padded
    114-stride row layout so all 9 taps become uniform AP offsets.
  - 7 taps run as float32r diag-matmuls on the PE accumulating in PSUM;
    2 taps run on DVE (tensor_scalar mult + scalar_tensor_tensor), merged
    with the PSUM partial by a final scalar_tensor_tensor.
  - PE transposes back to [spatial, c]; ACT evicts PSUM->SBUF; DMA out.
"""

import os
import sys

sys.path.insert(0, "/opt/trn_rl_repo")

import numpy as np

import concourse.bacc as bacc
import concourse.mybir as mybir
from concourse.tile import TileContext
from concourse.bass_utils import run_bass_kernel_spmd

F32 = mybir.dt.float32
F32R = mybir.dt.float32r

N_CORES = 8
B, H, W, C = 16, 112, 112, 384
IMG_PER_CORE = B // N_CORES          # 2
S = H * W                            # 12544 spatial positions per image
ROWS_PER_CORE = IMG_PER_CORE * S     # 25088
P = 128
CBLK = C // P                        # 3 channel blocks
WP = 114                             # padded row stride (w = -1 .. 112)
HP = 114                             # padded rows (h = -1 .. 112)
ROWG = 8                             # rows per transpose/evict group (8*112 = 7*128)
CHUNKS_PER_G = ROWG * W // P         # 7
NG = H // ROWG                       # 14 row groups per image
DMA_GROUPS = 7                       # input DMAs per (img, cblk): 16 rows each
ROWS_PER_DMA = H // DMA_GROUPS       # 16
CHUNKS_PER_DMA = ROWS_PER_DMA * W // P  # 14
TAP_ROWS = 4                         # output rows per tap matmul (N = 448)
NHG = H // TAP_ROWS                  # 28 tap groups per (img, cblk)

TAPS = [(dy, dx) for dy in (-1, 0, 1) for dx in (-1, 0, 1)]

# tunables (overridable via build_bass kwargs); defaults = best measured
# config from the TimelineSim sweep (333.7 us predicted vs 376 initial)
DEFAULT_CFG = dict(
    n_dve_taps=3,      # taps on DVE (rest on PE); int or per-hg list (cycled)
    act_first_mult=True,   # first DVE tap multiply on ACT instead of DVE
    out_dma_on_act=False,  # issue output DMAs on the ACT HWDGE ring
    tout_single=False,     # single 7-chunk T_out psum buffer + one big evict
    f32r_transpose=True,   # run PE transposes in float32r (1.5 vs 2 cyc/row)
    dve_inplace=False,     # DVE taps RMW directly into the tap PSUM bank;
                           # ACT evicts PSUM->staging (no DVE merge op)
    dve8=True,             # run DVE taps at 8-row granularity (amortize
                           # per-op overhead across two tap groups)
    tap_bufs=3,            # PSUM buffers for the tap accumulator (1 bank each)
    tout_bufs=3,           # PSUM buffers for the T_out stage
    tin_bufs=1,            # PSUM buffers for the T_in stage (2 banks each)
    xnat_bufs=4,
    stag_bufs=4,
    acc_bufs=4,
    onat_bufs=4,
    xt_bufs=2,
    in_dma_on_gpsimd=False,  # issue input DMAs on the SWDGE (gpsimd) path so
                             # the SP HWDGE ring carries only output DMAs
)


def _tap_idx(dy, dx):
    return (dy + 1) * 3 + (dx + 1)


def build_bass(reps=1, **cfg_over):
    cfg = {**DEFAULT_CFG, **cfg_over}
    tdt = F32R if cfg["f32r_transpose"] else F32
    nc = bacc.Bacc(
        "TRN2", target_bir_lowering=False, debug=False, num_devices=N_CORES
    )
    x_d = nc.dram_tensor("x", [ROWS_PER_CORE, C], tdt, kind="ExternalInput").ap()
    # float32r end-to-end for the PE-tap operands: the BIR verifier requires
    # every producer of fp32r-matmul data to round to fp32r.
    diag_d = nc.dram_tensor(
        "diag", [P, 9 * CBLK * P], F32R, kind="ExternalInput"
    ).ap()
    signs_d = nc.dram_tensor("signs", [P, 9 * CBLK], F32, kind="ExternalInput").ap()
    ident_d = nc.dram_tensor("ident", [P, P], tdt, kind="ExternalInput").ap()
    out_d = nc.dram_tensor("out", [ROWS_PER_CORE, C], tdt, kind="ExternalOutput").ap()

    with TileContext(nc) as tc:
        with (
            tc.tile_pool(name="const", bufs=1) as const_pool,
            tc.tile_pool(name="xnat", bufs=cfg["xnat_bufs"]) as xnat_pool,
            tc.tile_pool(name="xT", bufs=cfg["xt_bufs"]) as xT_pool,
            tc.tile_pool(name="acc", bufs=cfg["acc_bufs"]) as acc_pool,
            tc.tile_pool(name="stag", bufs=cfg["stag_bufs"]) as stag_pool,
            tc.tile_pool(name="onat", bufs=cfg["onat_bufs"]) as onat_pool,
            tc.tile_pool(
                name="tinp", bufs=cfg["tin_bufs"], space="PSUM"
            ) as tin_psum,
            tc.tile_pool(
                name="tapp", bufs=cfg["tap_bufs"], space="PSUM"
            ) as tap_psum,
            tc.tile_pool(
                name="toutp",
                bufs=1 if cfg["tout_single"] else cfg["tout_bufs"],
                space="PSUM",
            ) as tout_psum,
        ):
            diag_sb = const_pool.tile([P, 9 * CBLK * P], F32R)
            nc.sync.dma_start(diag_sb[:], diag_d)
            signs_sb = const_pool.tile([P, 9 * CBLK], F32)
            nc.sync.dma_start(signs_sb[:], signs_d)
            ident_sb = const_pool.tile([P, P], tdt)
            nc.sync.dma_start(ident_sb[:], ident_d)

            for _rep in range(reps):
                for img in range(IMG_PER_CORE):
                    for b in range(CBLK):
                        _unit(
                            nc, tc, img, b,
                            x_d, out_d, diag_sb, signs_sb, ident_sb,
                            xnat_pool, xT_pool, acc_pool, stag_pool, onat_pool,
                            tin_psum, tap_psum, tout_psum, cfg,
                        )
    nc.finalize()
    return nc


def _unit(
    nc, tc, img, b,
    x_d, out_d, diag_sb, signs_sb, ident_sb,
    xnat_pool, xT_pool, acc_pool, stag_pool, onat_pool,
    tin_psum, tap_psum, tout_psum, cfg,
):
    nd = cfg["n_dve_taps"]
    nd_list = [nd] * NHG if isinstance(nd, int) else [
        nd[i % len(nd)] for i in range(NHG)
    ]
    tdt = F32R if cfg["f32r_transpose"] else F32
    row0 = img * S

    # ---- transposed, zero-padded x for this (image, channel block) ----
    # Stored as float32r (the PE-tap moving operand must be fp32r-rounded by
    # its producers); DVE taps read it through a plain-f32 bitcast view.
    xT = xT_pool.tile([P, HP * WP], F32R, tag="xT")
    xT3 = xT.rearrange("p (r w) -> p r w", w=WP)  # [128, 114, 114]
    xT3f = xT.bitcast(F32).rearrange("p (r w) -> p r w", w=WP)
    # zero the pad slots: top pad row (h=-1), bottom pad row (h=112),
    # and the two w-pad slots of every row (contiguous pairs at 114k+113).
    # memset can't encode an f32r value in ISA -> write the zero bits as u32.
    xTu3 = xT.bitcast(mybir.dt.uint32).rearrange("p (r w) -> p r w", w=WP)
    nc.vector.memset(xTu3[:, 0, :], 0)
    nc.vector.memset(xTu3[:, HP - 1, :], 0)
    wpads = xT.bitcast(mybir.dt.uint32)[
        :, WP - 1 : WP - 1 + (HP - 1) * WP
    ].rearrange("p (r t) -> p r t", t=WP)[:, :, 0:2]
    nc.vector.memset(wpads, 0)

    # ---- load + transpose-in ----
    for dg in range(DMA_GROUPS):
        xnat = xnat_pool.tile([P, CHUNKS_PER_DMA, P], tdt, tag="xnat")
        src = x_d[
            row0 + dg * ROWS_PER_DMA * W : row0 + (dg + 1) * ROWS_PER_DMA * W,
            b * P : (b + 1) * P,
        ].rearrange("(k p) c -> p k c", p=P)
        if cfg["in_dma_on_gpsimd"]:
            nc.gpsimd.dma_start(xnat[:], src)
        else:
            nc.sync.dma_start(xnat[:], src)
        for half in range(2):
            g = dg * 2 + half  # row group (8 rows) index, 0..13
            ps_tin = tin_psum.tile([P, ROWG * W], tdt, tag="tin")
            for k in range(CHUNKS_PER_G):
                ck = half * CHUNKS_PER_G + k
                nc.tensor.transpose(
                    ps_tin[:, k * P : (k + 1) * P], xnat[:, ck, :], ident_sb[:]
                )
            # evict into padded rows 8g..8g+7 (padded row index 8g+1..8g+9)
            dst = xT3[:, g * ROWG + 1 : g * ROWG + 1 + ROWG, 1 : 1 + W]
            src_v = ps_tin.rearrange("p (r w) -> p r w", w=W)
            nc.scalar.copy(dst, src_v)

    # ---- taps ----
    stag = None
    for hg in range(NHG):
        n_dve = nd_list[hg]
        dve_taps = TAPS[9 - n_dve :]
        pe_taps = TAPS[: 9 - n_dve]
        h0 = hg * TAP_ROWS
        ps_acc = tap_psum.tile([P, TAP_ROWS * W], F32, tag="tap")
        for i, (dy, dx) in enumerate(pe_taps):
            t = _tap_idx(dy, dx)
            lhsT = diag_sb[:, (t * CBLK + b) * P : (t * CBLK + b + 1) * P]
            rhs = xT3[
                :, h0 + 1 + dy : h0 + 1 + dy + TAP_ROWS, 1 + dx : 1 + dx + W
            ]
            nc.tensor.matmul(
                ps_acc[:],
                lhsT,
                rhs,
                start=(i == 0),
                stop=(i == len(pe_taps) - 1),
            )
        if hg % 2 == 0:
            stag = stag_pool.tile([P, 2 * TAP_ROWS * W], tdt, tag="stag")
        half = hg % 2
        stag_slice = stag[:, half * TAP_ROWS * W : (half + 1) * TAP_ROWS * W]

        def dve_tap_views(i, rows=TAP_ROWS, hh=None):
            dy, dx = dve_taps[i]
            t = _tap_idx(dy, dx)
            sg = signs_sb[:, t * CBLK + b : t * CBLK + b + 1]
            hs = h0 if hh is None else hh
            xs = xT3f[
                :, hs + 1 + dy : hs + 1 + dy + rows, 1 + dx : 1 + dx + W
            ]
            return sg, xs

        if cfg["dve8"] and n_dve > 0:
            # 8-row DVE partial computed once per hg pair
            if half == 0:
                acc8 = acc_pool.tile([P, 2 * TAP_ROWS * W], F32, tag="acc")
                acc8v = acc8.rearrange("p (r w) -> p r w", w=W)
                _unit._acc8 = acc8  # stash on fn (single-threaded build)
                for i in range(n_dve):
                    sg, xs = dve_tap_views(i, rows=2 * TAP_ROWS)
                    if i == 0:
                        if cfg["act_first_mult"]:
                            nc.scalar.mul(acc8v, xs, sg)
                        else:
                            nc.vector.tensor_scalar(
                                acc8v, xs, sg, None, mybir.AluOpType.mult
                            )
                    else:
                        nc.vector.scalar_tensor_tensor(
                            acc8v, xs, sg, acc8v,
                            mybir.AluOpType.mult, mybir.AluOpType.add,
                        )
            acc8 = _unit._acc8
            nc.vector.scalar_tensor_tensor(
                stag_slice,
                ps_acc[:],
                1.0,
                acc8[:, half * TAP_ROWS * W : (half + 1) * TAP_ROWS * W],
                mybir.AluOpType.mult,
                mybir.AluOpType.add,
            )
        elif n_dve == 0:
            # no DVE partial: evict PSUM straight into staging on ACT
            nc.scalar.copy(stag_slice, ps_acc[:])
        elif cfg["dve_inplace"]:
            # DVE taps read-modify-write the PSUM accumulator after the PE
            # group completes; ACT does the final eviction into staging.
            for i in range(n_dve):
                sg, xs = dve_tap_views(i)
                nc.vector.scalar_tensor_tensor(
                    ps_acc[:], xs, sg, ps_acc[:],
                    mybir.AluOpType.mult, mybir.AluOpType.add,
                )
            nc.scalar.copy(stag_slice, ps_acc[:])
        else:
            acc = acc_pool.tile([P, TAP_ROWS * W], F32, tag="acc")
            accv = acc.rearrange("p (r w) -> p r w", w=W)
            for i in range(n_dve):
                sg, xs = dve_tap_views(i)
                if i == 0:
                    if cfg["act_first_mult"]:
                        nc.scalar.mul(accv, xs, sg)
                    else:
                        nc.vector.tensor_scalar(
                            accv, xs, sg, None, mybir.AluOpType.mult
                        )
                else:
                    nc.vector.scalar_tensor_tensor(
                        accv, xs, sg, accv,
                        mybir.AluOpType.mult, mybir.AluOpType.add,
                    )
            # merge PE partial (PSUM) + DVE partial into the staging tile
            nc.vector.scalar_tensor_tensor(
                stag_slice,
                ps_acc[:],
                1.0,
                acc[:],
                mybir.AluOpType.mult,
                mybir.AluOpType.add,
            )
        # ---- transpose-out + evict + store per 8 output rows ----
        if half == 1:
            g = hg // 2
            onat = onat_pool.tile([P, CHUNKS_PER_G, P], tdt, tag="onat")
            if cfg["tout_single"]:
                ps_out = tout_psum.tile([P, CHUNKS_PER_G * P], tdt, tag="tout")
                for k in range(CHUNKS_PER_G):
                    nc.tensor.transpose(
                        ps_out[:, k * P : (k + 1) * P],
                        stag[:, k * P : (k + 1) * P],
                        ident_sb[:],
                    )
                nc.scalar.copy(
                    onat[:],
                    ps_out[:].rearrange("p (k c) -> p k c", c=P),
                )
            else:
                for batch, nchunk in ((0, 4), (1, 3)):
                    ps_out = tout_psum.tile([P, 4 * P], tdt, tag="tout")
                    for k in range(nchunk):
                        ck = batch * 4 + k
                        nc.tensor.transpose(
                            ps_out[:, k * P : (k + 1) * P],
                            stag[:, ck * P : (ck + 1) * P],
                            ident_sb[:],
                        )
                    nc.scalar.copy(
                        onat[:, batch * 4 : batch * 4 + nchunk, :],
                        ps_out[:, : nchunk * P].rearrange("p (k c) -> p k c", c=P),
                    )
            dst = out_d[
                row0 + g * ROWG * W : row0 + (g + 1) * ROWG * W,
                b * P : (b + 1) * P,
            ].rearrange("(k p) c -> p k c", p=P)
            if cfg["out_dma_on_act"]:
                nc.scalar.dma_start(dst, onat[:])
            else:
                nc.sync.dma_start(dst, onat[:])


_NC_CACHE = None


def _get_nc():
    global _NC_CACHE
    if _NC_CACHE is None:
        _NC_CACHE = build_bass()
    return _NC_CACHE


def _host_inputs(w):
    """Per-core constant tensors derived from w (shared by all cores)."""
    signs = np.sign(np.clip(w.astype(np.float32), -1.0, 1.0))[:, :, :, 0]  # (3,3,384)
    signs_flat = signs.reshape(9, C)  # tap-major
    diag = np.zeros((P, 9 * CBLK * P), dtype=np.float32)
    signs_in = np.zeros((P, 9 * CBLK), dtype=np.float32)
    for t in range(9):
        for b in range(CBLK):
            sv = signs_flat[t, b * P : (b + 1) * P]
            col0 = (t * CBLK + b) * P
            diag[np.arange(P), col0 + np.arange(P)] = sv
            signs_in[:, t * CBLK + b] = sv
    ident = np.eye(P, dtype=np.float32)
    return diag, signs_in, ident


def kernel(x, w):
    x = np.asarray(x, dtype=np.float32)
    w = np.asarray(w, dtype=np.float32)
    assert x.shape == (B, H, W, C), x.shape
    nc = _get_nc()
    diag, signs_in, ident = _host_inputs(w)
    in_maps = []
    for core in range(N_CORES):
        xc = x[core * IMG_PER_CORE : (core + 1) * IMG_PER_CORE]
        in_maps.append(
            {
                "x": np.ascontiguousarray(xc.reshape(ROWS_PER_CORE, C)),
                "diag": diag,
                "signs": signs_in,
                "ident": ident,
            }
        )
    res = run_bass_kernel_spmd(nc, in_maps, core_ids=list(range(N_CORES)))
    out = np.empty((B, H, W, C), dtype=np.float32)
    for core in range(N_CORES):
        out[core * IMG_PER_CORE : (core + 1) * IMG_PER_CORE] = res.results[core][
            "out"
        ].reshape(IMG_PER_CORE, H, W, C)
    return out


if __name__ == "__main__":
    rng = np.random.default_rng(0)
    x = rng.standard_normal((B, H, W, C), dtype=np.float32)
    w = rng.standard_normal((3, 3, C, 1), dtype=np.float32)
    out = kernel(x, w)
    print("out", out.shape, out.dtype, float(np.abs(out).mean()))



# revision 5
# speedup vs baseline: 2.3038x; 2.3038x over previous
"""Binary depthwise 3x3 conv (SAME) on 8 Trainium2 NeuronCores.

Problem: x (16,112,112,384) f32, w (3,3,384,1) f32.
out[n,h,w,c] = sum_{dy,dx} sign(clip(w))[dy,dx,c] * x[n,h+dy-1,w+dx-1,c]

Strategy (data-parallel, 2 images per core), banded-stationary matmul:
  - Host packs x into bf16 tiles with partition dim = (h16, c8): 16
    consecutive (padded) image rows x 8 channels. Blocks advance by 14
    rows so each block computes 14 valid output rows (rows 0 and 15 of
    the block lack a neighbor and are discarded). The w dim is padded
    to 114 on the host; all zero-padding is baked in on the host, so
    the device does no transposes and no memsets.
  - For each channel-block cs (8 channels) and dx in {-1,0,1}, a single
    128x128 banded stationary matrix encodes all 3 dy taps x 8 channel
    signs: lhsT[(h_in,c),(h_out,c)] = sign[h_in-h_out+1, dx, c] for
    |h_in-h_out|<=1. Three matmuls (one per dx, rhs shifted along w)
    accumulate all 9 taps into PSUM.
  - ACT/DVE alternate evicting PSUM (f32) -> SBUF bf16; DMA out bf16.
  - Host unpacks bf16 -> f32 NHWC. Relative error ~1e-3 (bf16 rounding),
    well inside the 2e-2 gate, for ~2.3x less HBM traffic and ~2.6x
    less PE work than the f32r tap-matmul design.
"""

import sys

sys.path.insert(0, "/opt/trn_rl_repo")

import numpy as np
import ml_dtypes

import concourse.bacc as bacc
import concourse.mybir as mybir
from concourse.tile import TileContext
from concourse.bass_utils import run_bass_kernel_spmd

F32 = mybir.dt.float32
BF16 = mybir.dt.bfloat16

N_CORES = 8
B, H, W, C = 16, 112, 112, 384
IMG = B // N_CORES       # 2 images per core
P = 128
CL = 8                   # channels per stationary block
CS = C // CL             # 48 channel blocks
CS_PER_G = 4             # channel blocks per DMA group
CSG = CS // CS_PER_G     # 12 DMA groups
NB = 8                   # row blocks of 16 padded rows, advancing by 14
HB = 16                  # rows per block
ADV = 14                 # valid output rows per block
WP = 114                 # padded row width (w = -1 .. 112)
IN_COLS = CS_PER_G * NB * WP    # 3648
OUT_COLS = CS_PER_G * NB * W    # 3584
NST = CS * 3             # 144 stationary matrices

DEFAULT_CFG = dict(
    xin_bufs=3,
    osb_bufs=3,
    ps_bufs=4,
    evict_split=2,      # cs % N == 0 -> ACT else DVE
)


def build_bass(**cfg_over):
    cfg = {**DEFAULT_CFG, **cfg_over}
    nc = bacc.Bacc(
        "TRN2", target_bir_lowering=False, debug=False, num_devices=N_CORES
    )
    x_d = nc.dram_tensor(
        "x", [IMG * CSG * P, IN_COLS], BF16, kind="ExternalInput"
    ).ap()
    st_d = nc.dram_tensor("st", [P, NST * P], BF16, kind="ExternalInput").ap()
    out_d = nc.dram_tensor(
        "out", [IMG * CSG * 112, OUT_COLS], BF16, kind="ExternalOutput"
    ).ap()

    with TileContext(nc) as tc:
        with (
            tc.tile_pool(name="const", bufs=1) as const_pool,
            tc.tile_pool(name="xin", bufs=cfg["xin_bufs"]) as xin_pool,
            tc.tile_pool(name="osb", bufs=cfg["osb_bufs"]) as out_pool,
            tc.tile_pool(name="ps", bufs=cfg["ps_bufs"], space="PSUM") as psum_pool,
        ):
            st_sb = const_pool.tile([P, NST * P], BF16)
            ST_CHUNK = CS_PER_G * 3 * P  # stationaries for one csg
            for img in range(IMG):
                for csg in range(CSG):
                    if img == 0:
                        # stream stationaries per-csg on the ACT ring so the
                        # first input tile isn't stuck behind a 13us DMA
                        c0 = csg * ST_CHUNK
                        nc.scalar.dma_start(
                            st_sb[:, c0 : c0 + ST_CHUNK],
                            st_d[:, c0 : c0 + ST_CHUNK],
                        )
                    row0 = (img * CSG + csg) * P
                    xin = xin_pool.tile([P, IN_COLS], BF16, tag="xin")
                    nc.sync.dma_start(xin[:], x_d[row0 : row0 + P, :])
                    xv = xin.rearrange("p (g b w) -> p g b w", b=NB, w=WP)
                    osb = out_pool.tile([P, OUT_COLS], BF16, tag="osb")
                    osbv = osb.rearrange(
                        "p (g h q) -> p g h q", g=CS_PER_G, q=448
                    )
                    for g in range(CS_PER_G):
                        cs = csg * CS_PER_G + g
                        # [128, 1024] f32 = 2 PSUM banks; halves at 0 and 512
                        # so each 448-col matmul stays inside one bank.
                        ps = psum_pool.tile([P, 1024], F32, tag="ps")
                        for half in range(2):
                            for dxi in range(3):
                                st = st_sb[
                                    :, (cs * 3 + dxi) * P : (cs * 3 + dxi + 1) * P
                                ]
                                rhs = xv[
                                    :, g, half * 4 : half * 4 + 4, dxi : dxi + W
                                ]
                                nc.tensor.matmul(
                                    ps[:, half * 512 : half * 512 + 448],
                                    st,
                                    rhs,
                                    start=(dxi == 0),
                                    stop=(dxi == 2),
                                )
                        # evict all 128 partitions (engines can't start PSUM
                        # access at partition 8); the out-DMA selects 8..119.
                        src = ps.rearrange("p (h q) -> p h q", q=512)[
                            :, :, 0:448
                        ]
                        dst = osbv[:, g]
                        if cs % cfg["evict_split"] == 0:
                            nc.scalar.copy(dst, src)
                        else:
                            nc.vector.tensor_scalar(
                                dst, src, 1.0, None, mybir.AluOpType.mult
                            )
                    orow0 = (img * CSG + csg) * 112
                    # out-DMAs on the SWDGE (gpsimd) ring: keeps the SP ring
                    # free-flowing for input DMAs (no head-of-line blocking
                    # behind eviction waits)
                    nc.gpsimd.dma_start(
                        out_d[orow0 : orow0 + 112, :], osb[8:120, :]
                    )
    nc.finalize()
    return nc


_NC_CACHE = None


def _get_nc():
    global _NC_CACHE
    if _NC_CACHE is None:
        _NC_CACHE = build_bass()
    return _NC_CACHE


def _pack_x(xc):
    """(2,112,112,384) f32 -> [IMG*CSG*128, IN_COLS] bf16 banded layout."""
    xp = np.pad(xc, ((0, 0), (1, 1), (1, 1), (0, 0)))  # (2,114,114,384)
    rows = ADV * np.arange(NB)[:, None] + np.arange(HB)[None, :]  # (8,16)
    xb = xp[:, rows]  # (2, 8, 16, 114, 384)
    xb = xb.reshape(IMG, NB, HB, WP, CSG, CS_PER_G, CL)
    # (img, b, h, w, csg, g, cl) -> (img, csg, h, cl, g, b, w)
    arr = xb.transpose(0, 4, 2, 6, 5, 1, 3)
    return np.ascontiguousarray(arr).reshape(IMG * CSG * P, IN_COLS).astype(
        ml_dtypes.bfloat16
    )


def _stationaries(w):
    """w (3,3,384,1) -> [128, NST*128] bf16 banded sign matrices."""
    sgn = np.sign(np.clip(w.astype(np.float32), -1.0, 1.0))[:, :, :, 0]
    s = sgn.reshape(3, 3, CS, CL)  # (dy_tap, dxi, cs, cl)
    stat = np.zeros((P, NST, P), dtype=np.float32)
    cs_idx = np.arange(CS)[:, None] * 3 + np.arange(3)[None, :]  # (CS,3)
    for dy in (-1, 0, 1):
        for h_out in range(HB):
            h_in = h_out + dy
            if not 0 <= h_in < HB:
                continue
            for cl in range(CL):
                stat[h_in * CL + cl, cs_idx, h_out * CL + cl] = s[
                    dy + 1, :, :, cl
                ].T
    return stat.reshape(P, NST * P).astype(ml_dtypes.bfloat16)


def _unpack_out(o):
    """[IMG*CSG*112, OUT_COLS] bf16 -> (2,112,112,384) f32."""
    o = np.asarray(o).reshape(IMG, CSG, ADV, CL, CS_PER_G, NB, W)
    # (img, csg, h14, cl, g, b, w) -> (img, b, h14, w, csg, g, cl)
    full = o.transpose(0, 5, 2, 6, 1, 4, 3)
    return np.ascontiguousarray(full).reshape(IMG, H, W, C).astype(np.float32)


def kernel(x, w):
    x = np.asarray(x, dtype=np.float32)
    w = np.asarray(w, dtype=np.float32)
    assert x.shape == (B, H, W, C), x.shape
    nc = _get_nc()
    st = _stationaries(w)
    in_maps = []
    for core in range(N_CORES):
        xc = x[core * IMG : (core + 1) * IMG]
        in_maps.append({"x": _pack_x(xc), "st": st})
    res = run_bass_kernel_spmd(nc, in_maps, core_ids=list(range(N_CORES)))
    out = np.empty((B, H, W, C), dtype=np.float32)
    for core in range(N_CORES):
        out[core * IMG : (core + 1) * IMG] = _unpack_out(
            res.results[core]["out"]
        )
    return out


if __name__ == "__main__":
    rng = np.random.default_rng(0)
    x = rng.standard_normal((B, H, W, C), dtype=np.float32)
    w = rng.standard_normal((3, 3, C, 1), dtype=np.float32)
    out = kernel(x, w)
    print("out", out.shape, out.dtype, float(np.abs(out).mean()))
